# revision 1
# baseline (speedup 1.0000x reference)
"""Trainium2 Bass kernel for a dense transformer block (pre-LN, MHA + MLP).

Full inputs in, full outputs out. Sharding: 8 cores = (batch, seq-half).
Each core computes K/V over its batch element's full 1024 tokens and
Q/attention/MLP over its own 512 tokens (host permutes tokens so the core's
own half is always rows 0..511 — softmax over keys is permutation invariant).
No collectives needed.

Host-side preprocessing folds LayerNorm affine params into the following
matmul weights:  (xhat*g + b) @ W == xhat @ (diag(g) W) + b @ W.

All large on-chip buffers are split into per-slice tiles because the Tile
scheduler tracks dependencies at tile granularity — fine tiles let
consumers start as soon as their slice is ready.
"""

import sys

sys.path.insert(0, "/opt/trn_rl_repo")

import numpy as np

import concourse.bass as bass
import concourse.bacc as bacc
import concourse.mybir as mybir
import concourse.tile as tile
from concourse.bass_utils import run_bass_kernel_spmd
from concourse.masks import make_identity

P = 128
C = 1024
HEADS = 16
DH = 64
HID = 4096
NTOK = 1024  # tokens per batch element (kv length)
NOWN = 512  # tokens owned by this core (q length)
SCALE = DH ** -0.5
EPS = 1e-5

F32 = mybir.dt.float32
F32R = mybir.dt.float32r
BF16 = mybir.dt.bfloat16
AF = mybir.ActivationFunctionType
OP = mybir.AluOpType

CT = C // P  # 8 column tiles of the model dim
TT = NTOK // P  # 8 token tiles (kv)
QT = NOWN // P  # 4 token tiles (own)
HT = HID // P  # 32 hidden tiles

DEBUG_DUMPS = False  # set True to add per-phase debug outputs


def build_program():
    nc = bacc.Bacc("TRN2", target_bir_lowering=False)

    io = {}
    io["x"] = nc.dram_tensor("x", (NTOK, C), F32, kind="ExternalInput")
    io["qw"] = nc.dram_tensor("qw", (C, C), F32R, kind="ExternalInput")
    io["kw"] = nc.dram_tensor("kw", (C, C), F32R, kind="ExternalInput")
    io["vw"] = nc.dram_tensor("vw", (C, C), F32R, kind="ExternalInput")
    io["pw"] = nc.dram_tensor("pw", (C, C), F32R, kind="ExternalInput")
    io["f1w"] = nc.dram_tensor("f1w", (C, HID), F32R, kind="ExternalInput")
    io["f2w"] = nc.dram_tensor("f2w", (HID, C), BF16, kind="ExternalInput")
    # biases pre-transposed on host into [128, n] per-partition layout
    io["qbt"] = nc.dram_tensor("qbt", (P, CT), F32, kind="ExternalInput")
    io["kbt"] = nc.dram_tensor("kbt", (P, CT), F32, kind="ExternalInput")
    io["vbt"] = nc.dram_tensor("vbt", (P, CT), F32, kind="ExternalInput")
    io["f1bt"] = nc.dram_tensor("f1bt", (P, HT), F32, kind="ExternalInput")
    io["pb"] = nc.dram_tensor("pb", (C,), F32, kind="ExternalInput")
    io["f2b"] = nc.dram_tensor("f2b", (C,), F32, kind="ExternalInput")
    io["out"] = nc.dram_tensor("out", (NOWN, C), F32, kind="ExternalOutput")
    if DEBUG_DUMPS:
        io["d_hT"] = nc.dram_tensor(
            "d_hT", (P, CT, NTOK), F32, kind="ExternalOutput"
        )
        io["d_kT"] = nc.dram_tensor(
            "d_kT", (P, CT, NTOK), F32, kind="ExternalOutput"
        )
        io["d_qT"] = nc.dram_tensor(
            "d_qT", (P, CT, NOWN), F32, kind="ExternalOutput"
        )
        io["d_vh"] = nc.dram_tensor(
            "d_vh", (P, HEADS, TT, DH + 1), BF16, kind="ExternalOutput"
        )
        io["d_oT"] = nc.dram_tensor(
            "d_oT", (P, CT, NOWN), F32, kind="ExternalOutput"
        )
        io["d_x2"] = nc.dram_tensor(
            "d_x2", (P, QT, C), F32, kind="ExternalOutput"
        )
        io["d_h2T"] = nc.dram_tensor(
            "d_h2T", (P, CT, NOWN), F32, kind="ExternalOutput"
        )
        io["d_actT"] = nc.dram_tensor(
            "d_actT", (P, HT, NOWN), BF16, kind="ExternalOutput"
        )

    with tile.TileContext(nc) as tc:
        _emit(nc, tc, io)
    nc.compile()
    return nc


def _emit(nc, tc, io):
    x_d, out_d = io["x"], io["out"]

    with (
        tc.tile_pool(name="consts", bufs=1) as consts,
        tc.tile_pool(name="persist", bufs=1) as persist,
        tc.tile_pool(name="big", bufs=1) as big,
        tc.tile_pool(name="psum_tr", bufs=2, space="PSUM") as psum_tr,
    ):
        # ---- constants ----
        ident_f32 = consts.tile([P, P], F32)
        make_identity(nc, ident_f32)
        ident = consts.tile([P, P], F32R)
        nc.vector.tensor_copy(out=ident, in_=ident_f32)
        eps_tile = consts.tile([P, 1], F32)
        nc.vector.memset(eps_tile, EPS)
        qbT = consts.tile([P, CT], F32)
        nc.sync.dma_start(qbT, io["qbt"][:, :])
        kbT = consts.tile([P, CT], F32)
        nc.sync.dma_start(kbT, io["kbt"][:, :])
        vbT = consts.tile([P, CT], F32)
        nc.sync.dma_start(vbT, io["vbt"][:, :])
        f1bT = consts.tile([P, HT], F32)
        nc.sync.dma_start(f1bT, io["f1bt"][:, :])

        def bcast_const(src_d, n):
            t = consts.tile([P, n], F32)
            src = bass.AP(tensor=src_d, offset=0, ap=[[0, P], [1, n]])
            nc.sync.dma_start(t, src)
            return t

        pb_bc = bcast_const(io["pb"], C)
        f2b_bc = bcast_const(io["f2b"], C)

        # own x tiles (fp32, kept for the residual), one tile per token tile
        x_own = []
        for t in range(QT):
            xo = persist.tile([P, C], F32, tag=f"xo{t}", name=f"xo{t}")
            nc.sync.dma_start(xo, x_d[t * P : (t + 1) * P, :])
            x_own.append(xo)
        x2 = [
            persist.tile([P, C], F32, tag=f"x2_{t}", name=f"x2_{t}")
            for t in range(QT)
        ]

        def layernorm_tile(temps, xt):
            """xt: [128, C] fp32 -> returns normalized f32r tile [128, C]."""
            stats = temps.tile([P, 2, 6], F32, tag="ln_stats", name="st")
            for sg in range(2):
                nc.vector.bn_stats(
                    out=stats[:, sg, :], in_=xt[:, sg * 512 : (sg + 1) * 512]
                )
            mv = temps.tile([P, 2], F32, tag="ln_mv", name="mv")
            nc.vector.bn_aggr(out=mv[:], in_=stats[:])
            rstd = temps.tile([P, 1], F32, tag="ln_rstd", name="rstd")
            nc.scalar.activation(
                out=rstd, in_=mv[:, 1:2], func=AF.Sqrt, bias=eps_tile, scale=1.0
            )
            nc.vector.reciprocal(out=rstd, in_=rstd)
            nmr = temps.tile([P, 1], F32, tag="ln_nmr", name="nmr")
            nc.vector.tensor_tensor(nmr, mv[:, 0:1], rstd, OP.mult)
            nc.vector.tensor_scalar_mul(nmr, nmr, -1.0)
            h = temps.tile([P, C], F32R, tag="ln_h", name="h")
            nc.vector.tensor_scalar(
                out=h,
                in0=xt,
                scalar1=rstd,
                scalar2=nmr,
                op0=OP.mult,
                op1=OP.add,
            )
            return h

        def transpose_into(dst_view, src_view):
            """dst_view [128, 128] (f32r) <- transpose of src_view."""
            ps = psum_tr.tile([P, P], F32R, tag="tr", name="tr")
            nc.tensor.transpose(ps, src_view, ident)
            nc.any.tensor_copy(out=dst_view, in_=ps)

        # ---- per-slice phase buffers (tag-shared slots, serial reuse) ----
        # hT[(c, t2)]: [P, 512] f32r; slots reused later by actT (tag A*)
        hT = {
            (c, t2): big.tile(
                [P, 512], F32R, tag=f"A{(c * 2 + t2) % 16}", name=f"hT{c}_{t2}"
            )
            for c in range(CT)
            for t2 in range(2)
        }
        # kT[(ft, t2)]: [P, 512] f32r; slots reused later by h2T (tag B*)
        kT = {
            (ft, t2): big.tile(
                [P, 512], BF16, tag=f"B{(ft * 2 + t2) % 16}", name=f"kT{ft}_{t2}"
            )
            for ft in range(CT)
            for t2 in range(2)
        }
        # vh[h]: [P, TT, DH+1] bf16 head-padded V; V-tags reused by f2w groups
        vh = [
            big.tile([P, TT, P], BF16, tag=f"V{h}", name=f"vh{h}")
            for h in range(HEADS)
        ]
        qT = [
            big.tile([P, 512], BF16, tag=f"D{ft}", name=f"qT{ft}")
            for ft in range(CT)
        ]
        oT = [
            big.tile([P, 512], F32R, tag=f"E{ft}", name=f"oT{ft}")
            for ft in range(CT)
        ]

        # ================= Phase 1: LN1 -> hT =================
        with (
            tc.tile_pool(name="ln1", bufs=3) as ln1,
            tc.tile_pool(name="xtmp", bufs=3) as xtmp,
        ):
            for t in range(TT):
                if t < QT:
                    xt = x_own[t]
                else:
                    xt = xtmp.tile([P, C], F32, tag="xt", name="xt")
                    nc.sync.dma_start(xt, x_d[t * P : (t + 1) * P, :])
                h = layernorm_tile(ln1, xt)
                t2, tb = t // QT, t % QT
                for ft in range(CT):
                    transpose_into(
                        hT[(ft, t2)][:, tb * P : (tb + 1) * P],
                        h[:, ft * P : (ft + 1) * P],
                    )

        # ================= Phase 2: QKV =================
        for h in range(HEADS):
            nc.vector.memset(vh[h][:], 0.0)
            nc.vector.memset(vh[h][:, :, DH : DH + 1], 1.0)
        with (
            tc.tile_pool(name="wchunk", bufs=4) as wpool,
            tc.tile_pool(name="qkv_psum", bufs=4, space="PSUM") as qkv_psum,
        ):

            def kv_sweep(t2, w_d, bT, which):
                for ft in range(CT):
                    slab = wpool.tile([P, CT, P], F32R, tag="w_kv", name="slab")
                    nc.sync.dma_start(
                        slab,
                        w_d[:, ft * P : (ft + 1) * P].rearrange(
                            "(c p) f -> p c f", p=P
                        ),
                    )
                    ps = qkv_psum.tile([P, 512], F32, tag="kvps", name="kvps")
                    for c in range(CT):
                        nc.tensor.matmul(
                            ps,
                            lhsT=slab[:, c, :],
                            rhs=hT[(c, t2)],
                            start=(c == 0),
                            stop=(c == CT - 1),
                        )
                    if which == "k":
                        nc.vector.tensor_scalar(
                            out=kT[(ft, t2)],
                            in0=ps,
                            scalar1=bT[:, ft : ft + 1],
                            scalar2=None,
                            op0=OP.add,
                        )
                    else:
                        # vT tile [feat128, tok512] + bias; transpose 128x128
                        # blocks into head-padded vh layout.
                        vt = wpool.tile([P, 512], F32R, tag="vt_sb", name="vt")
                        nc.vector.tensor_scalar(
                            out=vt,
                            in0=ps,
                            scalar1=bT[:, ft : ft + 1],
                            scalar2=None,
                            op0=OP.add,
                        )
                        for b in range(4):
                            t = t2 * 4 + b
                            ps2 = psum_tr.tile([P, P], F32R, tag="tr", name="tr")
                            nc.tensor.transpose(
                                ps2, vt[:, b * P : (b + 1) * P], ident
                            )
                            # feat rows ft*128..: heads 2*ft, 2*ft+1
                            for hh in range(2):
                                nc.any.tensor_copy(
                                    out=vh[2 * ft + hh][:, t, :DH],
                                    in_=ps2[:, hh * DH : (hh + 1) * DH],
                                )

            # token half 0 only needs LN of tiles 0-3; K/V half 0 and Q can
            # overlap with LN of tiles 4-7.
            kv_sweep(0, io["kw"], kbT, "k")
            kv_sweep(0, io["vw"], vbT, "v")
            for ft in range(CT):
                slab = wpool.tile([P, CT, P], F32R, tag="w_kv", name="qslab")
                nc.sync.dma_start(
                    slab,
                    io["qw"][:, ft * P : (ft + 1) * P].rearrange(
                        "(c p) f -> p c f", p=P
                    ),
                )
                ps = qkv_psum.tile([P, 512], F32, tag="kvps", name="qps")
                for c in range(CT):
                    nc.tensor.matmul(
                        ps,
                        lhsT=slab[:, c, :],
                        rhs=hT[(c, 0)],
                        start=(c == 0),
                        stop=(c == CT - 1),
                    )
                nc.vector.tensor_scalar(
                    out=qT[ft],
                    in0=ps,
                    scalar1=qbT[:, ft : ft + 1],
                    scalar2=None,
                    op0=OP.add,
                )
            kv_sweep(1, io["kw"], kbT, "k")
            kv_sweep(1, io["vw"], vbT, "v")

        if DEBUG_DUMPS:
            for (c, t2), t_ in hT.items():
                nc.sync.dma_start(
                    io["d_hT"][:, c, t2 * 512 : (t2 + 1) * 512],
                    t_[:].bitcast(F32),
                )
            for (ft, t2), t_ in kT.items():
                nc.sync.dma_start(
                    io["d_kT"][:, ft, t2 * 512 : (t2 + 1) * 512],
                    t_[:].bitcast(F32),
                )
            for ft in range(CT):
                nc.sync.dma_start(io["d_qT"][:, ft, :], qT[ft][:].bitcast(F32))
            for h in range(HEADS):
                nc.sync.dma_start(io["d_vh"][:, h, :, :], vh[h][:])

        # ================= Phase 3: attention =================
        with (
            tc.tile_pool(name="attn", bufs=2) as attn_pool,
            tc.tile_pool(name="attn_st", bufs=3, space="PSUM") as attn_st,
            tc.tile_pool(name="attn_ot", bufs=2, space="PSUM") as attn_ot,
        ):
            for h in range(HEADS):
                prow = (h % 2) * DH
                ftile = h // 2
                p_sb = attn_pool.tile([P, TT, NOWN], BF16, tag="p_sb", name="p")
                for c in range(TT):
                    kv_slice = kT[(ftile, c // 4)][
                        prow : prow + DH, (c % 4) * P : (c % 4 + 1) * P
                    ]
                    st = attn_st.tile([P, 512], F32, tag="st", name="st")
                    nc.tensor.matmul(
                        st,
                        lhsT=kv_slice,
                        rhs=qT[ftile][prow : prow + DH, :],
                        start=True,
                        stop=True,
                    )
                    # p = exp(SCALE * s)   (bf16 out)
                    nc.scalar.activation(
                        out=p_sb[:, c, :], in_=st, func=AF.Exp, scale=SCALE
                    )
                ot = attn_ot.tile([P, 512], F32, tag="ot", name="ot")
                for c in range(TT):
                    nc.tensor.matmul(
                        ot,
                        lhsT=vh[h][:, c, :],
                        rhs=p_sb[:, c, :],
                        start=(c == 0),
                        stop=(c == TT - 1),
                    )
                # softmax denominators arrive in row DH (ones column of vh)
                rs = attn_pool.tile([1, NOWN], F32, tag="rs", name="rs")
                nc.vector.reciprocal(out=rs, in_=ot[DH : DH + 1, :])
                rsb = attn_pool.tile([DH, NOWN], F32, tag="rsb", name="rsb")
                nc.gpsimd.partition_broadcast(rsb, rs)
                nc.vector.tensor_tensor(
                    oT[ftile][prow : prow + DH, :], ot[:DH, :], rsb, OP.mult
                )

        if DEBUG_DUMPS:
            for ft in range(CT):
                nc.sync.dma_start(io["d_oT"][:, ft, :], oT[ft][:].bitcast(F32))

        # ================= Phase 4: proj + residual -> x2 =================
        with (
            tc.tile_pool(name="pwc", bufs=2) as pwc,
            tc.tile_pool(name="proj_ps", bufs=1, space="PSUM") as proj_ps,
        ):
            for ns in range(2):
                nsl = slice(ns * 512, (ns + 1) * 512)
                pss = [
                    proj_ps.tile([P, 512], F32, tag=f"pps{tq}", name=f"pps{tq}")
                    for tq in range(QT)
                ]
                for fh in range(2):
                    slab = pwc.tile([P, 4, 512], F32R, tag="pw", name="pwslab")
                    nc.sync.dma_start(
                        slab,
                        io["pw"][fh * 512 : (fh + 1) * 512, nsl].rearrange(
                            "(c p) n -> p c n", p=P
                        ),
                    )
                    for c in range(4):
                        f = fh * 4 + c
                        for tq in range(QT):
                            nc.tensor.matmul(
                                pss[tq],
                                lhsT=oT[f][:, tq * P : (tq + 1) * P],
                                rhs=slab[:, c, :],
                                start=(f == 0),
                                stop=(f == CT - 1),
                            )
                for tq in range(QT):
                    nc.vector.tensor_add(pss[tq], pss[tq], pb_bc[:, nsl])
                    nc.vector.tensor_add(
                        x2[tq][:, nsl], pss[tq], x_own[tq][:, nsl]
                    )

        if DEBUG_DUMPS:
            for tq in range(QT):
                nc.sync.dma_start(io["d_x2"][:, tq, :], x2[tq][:])

        # ================= Phase 5: LN2 -> h2T (reuses kT slots) ==========
        h2T = [
            big.tile([P, 512], F32R, tag=f"B{c}", name=f"h2T{c}")
            for c in range(CT)
        ]
        with tc.tile_pool(name="ln2", bufs=3) as ln2:
            for t in range(QT):
                h = layernorm_tile(ln2, x2[t])
                for ft in range(CT):
                    transpose_into(
                        h2T[ft][:, t * P : (t + 1) * P],
                        h[:, ft * P : (ft + 1) * P],
                    )

        if DEBUG_DUMPS:
            for ft in range(CT):
                nc.sync.dma_start(
                    io["d_h2T"][:, ft, :], h2T[ft][:].bitcast(F32)
                )

        # ================= Phase 6: FC1 + gelu -> actT (reuses hT slots) ==
        def _act_tag(hf):
            if hf < 16:
                return f"A{hf}"
            if hf < 24:
                return f"D{hf - 16}"
            return f"E{hf - 24}"

        actT = [
            big.tile([P, 512], BF16, tag=_act_tag(hf), name=f"actT{hf}")
            for hf in range(HT)
        ]
        with (
            tc.tile_pool(name="f1c", bufs=4) as f1c,
            tc.tile_pool(name="f1_ps", bufs=4, space="PSUM") as f1_ps,
        ):
            for hf in range(HT):
                ps = f1_ps.tile([P, 512], F32, tag="f1ps", name="f1ps")
                slab = f1c.tile([P, CT, P], F32R, tag="f1w", name="f1slab")
                nc.sync.dma_start(
                    slab,
                    io["f1w"][:, hf * P : (hf + 1) * P].rearrange(
                        "(c p) f -> p c f", p=P
                    ),
                )
                for c in range(CT):
                    nc.tensor.matmul(
                        ps,
                        lhsT=slab[:, c, :],
                        rhs=h2T[c],
                        start=(c == 0),
                        stop=(c == CT - 1),
                    )
                # gelu(ps + f1b), fused bias via activation
                nc.scalar.activation(
                    out=actT[hf],
                    in_=ps,
                    func=AF.Gelu,
                    bias=f1bT[:, hf : hf + 1],
                    scale=1.0,
                )

        if DEBUG_DUMPS:
            for hf in range(HT):
                nc.sync.dma_start(io["d_actT"][:, hf, :], actT[hf][:])

        # ================= Phase 7: FC2 + residual -> out =================
        # f2w streamed in 512KB groups of 4 hidden-tiles (reuses vh V-tags)
        with (
            tc.tile_pool(name="f2_ps", bufs=4, space="PSUM") as f2_ps,
            tc.tile_pool(name="out_sb", bufs=2) as out_pool,
        ):
            for ns in range(2):
                nsl = slice(ns * 512, (ns + 1) * 512)
                groups = []
                for g in range(8):
                    gw = big.tile(
                        [P, 4, 512], BF16, tag=f"V{g}", name=f"f2wg{g}_{ns}"
                    )
                    nc.sync.dma_start(
                        gw,
                        io["f2w"][g * 512 : (g + 1) * 512, nsl].rearrange(
                            "(o p) n -> p o n", p=P
                        ),
                    )
                    groups.append(gw)
                for tq in range(QT):
                    ps = f2_ps.tile([P, 512], F32, tag="f2ps", name="f2ps")
                    for hc in range(HT):
                        nc.tensor.matmul(
                            ps,
                            lhsT=actT[hc][:, tq * P : (tq + 1) * P],
                            rhs=groups[hc // 4][:, hc % 4, :],
                            start=(hc == 0),
                            stop=(hc == HT - 1),
                        )
                    ot2 = out_pool.tile([P, 512], F32, tag="out_t", name="o")
                    nc.vector.tensor_add(ps, ps, f2b_bc[:, nsl])
                    nc.vector.tensor_add(ot2, ps, x2[tq][:, nsl])
                    nc.sync.dma_start(out_d[tq * P : (tq + 1) * P, nsl], ot2)


_PROGRAM = None


def _get_program():
    global _PROGRAM
    if _PROGRAM is None:
        _PROGRAM = build_program()
    return _PROGRAM


def build_in_maps(inputs):
    x = np.asarray(inputs["x"], np.float32)  # [4, 1024, 1024]
    ln1_g = np.asarray(inputs["ln1_g"], np.float64)
    ln1_b = np.asarray(inputs["ln1_b"], np.float64)
    ln2_g = np.asarray(inputs["ln2_g"], np.float64)
    ln2_b = np.asarray(inputs["ln2_b"], np.float64)
    qkv_w = np.asarray(inputs["qkv_w"], np.float64)
    qkv_b = np.asarray(inputs["qkv_b"], np.float64)
    proj_w = np.asarray(inputs["proj_w"], np.float32)
    proj_b = np.asarray(inputs["proj_b"], np.float32)
    fc1_w = np.asarray(inputs["fc1_w"], np.float64)
    fc1_b = np.asarray(inputs["fc1_b"], np.float64)
    fc2_w = np.asarray(inputs["fc2_w"], np.float32)
    fc2_b = np.asarray(inputs["fc2_b"], np.float32)

    # Fold LN affine into the following matmul:
    #   (xhat*g + b) @ W == xhat @ (diag(g) W) + b @ W
    qkv_w_f = (ln1_g[:, None] * qkv_w).astype(np.float32)
    qkv_b_f = (qkv_b + ln1_b @ qkv_w).astype(np.float32)
    f1w_f = (ln2_g[:, None] * fc1_w).astype(np.float32)
    f1b_f = (fc1_b + ln2_b @ fc1_w).astype(np.float32)

    qw = np.ascontiguousarray(qkv_w_f[:, :C])
    kw = np.ascontiguousarray(qkv_w_f[:, C : 2 * C])
    vw = np.ascontiguousarray(qkv_w_f[:, 2 * C :])

    def tbias(b):  # [n*128] -> [128, n] per-partition layout
        return np.ascontiguousarray(b.reshape(-1, P).T)

    import ml_dtypes

    f2w_bf = fc2_w.astype(ml_dtypes.bfloat16)

    common = dict(
        qw=qw, kw=kw, vw=vw, pw=proj_w, f1w=f1w_f, f2w=f2w_bf,
        qbt=tbias(qkv_b_f[:C]),
        kbt=tbias(qkv_b_f[C : 2 * C]),
        vbt=tbias(qkv_b_f[2 * C :]),
        f1bt=tbias(f1b_f),
        pb=proj_b, f2b=fc2_b,
    )
    in_maps = []
    for core in range(8):
        b, half = core // 2, core % 2
        own = x[b, half * NOWN : (half + 1) * NOWN, :]
        other = x[b, (1 - half) * NOWN : (2 - half) * NOWN, :]
        xp = np.ascontiguousarray(np.concatenate([own, other], axis=0))
        in_maps.append({**common, "x": xp})
    return in_maps


def kernel(**inputs):
    in_maps = build_in_maps(inputs)
    nc = _get_program()
    res = run_bass_kernel_spmd(nc, in_maps, core_ids=list(range(8)))
    outs = res.results

    y = np.empty((4, NTOK, C), np.float32)
    for core in range(8):
        b, half = core // 2, core % 2
        y[b, half * NOWN : (half + 1) * NOWN, :] = outs[core]["out"]
    return y


if __name__ == "__main__":
    prog = build_program()
    print("program built OK")



# revision 7
# speedup vs baseline: 1.3400x; 1.3400x over previous
"""Trainium2 Bass kernel for a dense transformer block (pre-LN, MHA + MLP).

Full inputs in, full outputs out. Sharding: 8 cores = (batch, seq-half).
Each core computes K/V over its batch element's full 1024 tokens and
Q/attention/MLP over its own 512 tokens (host permutes tokens so the core's
own half is always rows 0..511 — softmax over keys is permutation invariant).
No collectives needed.

v2: fp8 (e4m3) DoubleRow matmuls for QKV / AV / proj (and optionally
fc1/fc2), which stream 2 contraction rows per PE pass. Weights are
quantized per-output-column on the host (absmax -> +-240); dequant scales
fold into the existing bias-add / activation ops, or (for V) into the proj
weights themselves. V is computed in natural [token, feat] layout directly
(h stationary, weights moving), eliminating the separate V transpose pass.
The softmax denominator comes from a constant ones-column appended to V.

Host-side preprocessing folds LayerNorm affine params into the following
matmul weights:  (xhat*g + b) @ W == xhat @ (diag(g) W) + b @ W, and the
V bias into the proj bias: (o/d + vb) @ pw + pb == (o/d) @ pw + (vb@pw + pb).
"""

import sys

sys.path.insert(0, "/opt/trn_rl_repo")

import numpy as np

import concourse.bass as bass
import concourse.bacc as bacc
import concourse.mybir as mybir
import concourse.tile as tile
from concourse.bass_utils import run_bass_kernel_spmd
from concourse.masks import make_identity

P = 128
C = 1024
HEADS = 16
DH = 64
HID = 4096
NTOK = 1024  # tokens per batch element (kv length)
NOWN = 512  # tokens owned by this core (q length)
SCALE = DH ** -0.5
EPS = 1e-5

F32 = mybir.dt.float32
F32R = mybir.dt.float32r
BF16 = mybir.dt.bfloat16
F8 = mybir.dt.float8e4
AF = mybir.ActivationFunctionType
OP = mybir.AluOpType
DROW = mybir.MatmulPerfMode.DoubleRow

CT = C // P  # 8 column tiles of the model dim
CP = CT // 2  # 4 column-tile pairs
TT = NTOK // P  # 8 token tiles (kv)
QT = NOWN // P  # 4 token tiles (own)
HT = HID // P  # 32 hidden tiles
HP = HT // 2  # 16 hidden-tile pairs

VPAD = 72  # per-head padded width of the V tile (DH + ones col + pad)

E4M3_MAX = 240.0

# --- dtype config for the two MLP GEMMs (attention GEMMs are always fp8;
# the error sim shows attention fp8 contributes ~nothing to final error) ---
FC1_FP8 = False
FC2_FP8 = False


def build_program():
    nc = bacc.Bacc("TRN2", target_bir_lowering=False)
    mf1 = F8 if FC1_FP8 else BF16
    mf2 = F8 if FC2_FP8 else BF16

    io = {}
    io["x"] = nc.dram_tensor("x", (NTOK, C), F32, kind="ExternalInput")
    # pre-permuted weights (host layout matches SBUF slabs)
    io["qw"] = nc.dram_tensor("qw", (P, CT, CT, P), F8, kind="ExternalInput")
    io["kw"] = nc.dram_tensor("kw", (P, CT, CT, P), F8, kind="ExternalInput")
    io["vw"] = nc.dram_tensor("vw", (P, CT, C), F8, kind="ExternalInput")
    io["pw"] = nc.dram_tensor("pw", (P, CT, C), F8, kind="ExternalInput")
    io["f1w"] = nc.dram_tensor("f1w", (P, HT, CT, P), mf1, kind="ExternalInput")
    io["f2w"] = nc.dram_tensor("f2w", (P, 2, HT, NOWN), mf2, kind="ExternalInput")
    # per-partition bias/scale tables, [128, n] layouts
    io["qbt"] = nc.dram_tensor("qbt", (P, CT), F32, kind="ExternalInput")
    io["kbt"] = nc.dram_tensor("kbt", (P, CT), F32, kind="ExternalInput")
    io["qst"] = nc.dram_tensor("qst", (P, CT), F32, kind="ExternalInput")
    io["kst"] = nc.dram_tensor("kst", (P, CT), F32, kind="ExternalInput")
    io["f1bt"] = nc.dram_tensor("f1bt", (P, HT), F32, kind="ExternalInput")
    io["f1st"] = nc.dram_tensor("f1st", (P, HT), F32, kind="ExternalInput")
    # free-dim vectors (broadcast across partitions on chip)
    io["vg"] = nc.dram_tensor("vg", (C,), F32, kind="ExternalInput")
    io["psinv"] = nc.dram_tensor("psinv", (C,), F32, kind="ExternalInput")
    io["pb"] = nc.dram_tensor("pb", (C,), F32, kind="ExternalInput")
    io["f2sinv"] = nc.dram_tensor("f2sinv", (C,), F32, kind="ExternalInput")
    io["f2b"] = nc.dram_tensor("f2b", (C,), F32, kind="ExternalInput")
    io["out"] = nc.dram_tensor("out", (NOWN, C), F32, kind="ExternalOutput")

    with tile.TileContext(nc) as tc:
        _emit(nc, tc, io)
    nc.compile()
    return nc


def _emit(nc, tc, io):
    x_d, out_d = io["x"], io["out"]
    mf1 = F8 if FC1_FP8 else BF16
    mf2 = F8 if FC2_FP8 else BF16

    with (
        tc.tile_pool(name="consts", bufs=1) as consts,
        tc.tile_pool(name="persist", bufs=1) as persist,
        tc.tile_pool(name="big", bufs=1) as big,
        tc.tile_pool(name="psum_wide", bufs=2, space="PSUM") as psum_wide,
    ):
        # ---- constants (unique tags: each gets its own persistent slot) ----
        ident_f32 = consts.tile([P, P], F32, tag="idf")
        make_identity(nc, ident_f32)
        ident = consts.tile([P, P], F32R, tag="idr")
        nc.vector.tensor_copy(out=ident, in_=ident_f32)
        eps_tile = consts.tile([P, 1], F32, tag="eps")
        nc.vector.memset(eps_tile, EPS)
        qbT = consts.tile([P, CT], F32, tag="qbT")
        nc.sync.dma_start(qbT, io["qbt"][:, :])
        kbT = consts.tile([P, CT], F32, tag="kbT")
        nc.sync.dma_start(kbT, io["kbt"][:, :])
        qsT = consts.tile([P, CT], F32, tag="qsT")
        nc.sync.dma_start(qsT, io["qst"][:, :])
        ksT = consts.tile([P, CT], F32, tag="ksT")
        nc.sync.dma_start(ksT, io["kst"][:, :])
        f1bT = consts.tile([P, HT], F32, tag="f1bT")
        nc.sync.dma_start(f1bT, io["f1bt"][:, :])
        f1sT = consts.tile([P, HT], F32, tag="f1sT")
        nc.sync.dma_start(f1sT, io["f1st"][:, :])

        def bcast_const(src_d, n, tag):
            t = consts.tile([P, n], F32, tag=tag, name=tag)
            src = bass.AP(tensor=src_d, offset=0, ap=[[0, P], [1, n]])
            nc.sync.dma_start(t, src)
            return t

        vg_bc = bcast_const(io["vg"], C, "vg")
        psinv_bc = bcast_const(io["psinv"], C, "psv")
        pb_bc = bcast_const(io["pb"], C, "pbb")
        f2sinv_bc = bcast_const(io["f2sinv"], C, "f2s")
        f2b_bc = bcast_const(io["f2b"], C, "f2bb")

        # own x tiles (fp32, kept for the residual), one tile per token tile;
        # proj writes x2 = x + pb + proj_out back IN PLACE (saves SBUF)
        x_own = []
        for t in range(QT):
            xo = persist.tile([P, C], F32, tag=f"xo{t}", name=f"xo{t}")
            nc.sync.dma_start(xo, x_d[t * P : (t + 1) * P, :])
            x_own.append(xo)
        x2 = x_own

        # persistent weight slabs (single DMA each, reused across sweeps)
        kwslab = persist.tile([P, CT, CT, P], F8, tag="kws", name="kws")
        nc.sync.dma_start(kwslab, io["kw"][:, :, :, :])
        qwslab = persist.tile([P, CT, CT, P], F8, tag="qws", name="qws")
        nc.sync.dma_start(qwslab, io["qw"][:, :, :, :])
        vwslab = persist.tile([P, CT, C], F8, tag="vws", name="vws")
        nc.sync.dma_start(vwslab, io["vw"][:, :, :])
        pslab = persist.tile([P, CT, C], F8, tag="pws", name="pws")
        nc.sync.dma_start(pslab, io["pw"][:, :, :])

        def layernorm_tile(temps, xt):
            """xt: [128, C] fp32 -> returns normalized f32r tile [128, C]."""
            stats = temps.tile([P, 2, 6], F32, tag="ln_stats", name="st")
            for sg in range(2):
                nc.vector.bn_stats(
                    out=stats[:, sg, :], in_=xt[:, sg * 512 : (sg + 1) * 512]
                )
            mv = temps.tile([P, 2], F32, tag="ln_mv", name="mv")
            nc.vector.bn_aggr(out=mv[:], in_=stats[:])
            rstd = temps.tile([P, 1], F32, tag="ln_rstd", name="rstd")
            nc.scalar.activation(
                out=rstd, in_=mv[:, 1:2], func=AF.Sqrt, bias=eps_tile, scale=1.0
            )
            nc.vector.reciprocal(out=rstd, in_=rstd)
            nmr = temps.tile([P, 1], F32, tag="ln_nmr", name="nmr")
            nc.vector.tensor_tensor(nmr, mv[:, 0:1], rstd, OP.mult)
            nc.vector.tensor_scalar_mul(nmr, nmr, -1.0)
            h = temps.tile([P, C], F32R, tag="ln_h", name="h")
            nc.vector.tensor_scalar(
                out=h,
                in0=xt,
                scalar1=rstd,
                scalar2=nmr,
                op0=OP.mult,
                op1=OP.add,
            )
            return h

        # ---- persistent activation tiles ----
        # hT2[(cp, t2)]: [P, 2, 512] fp8 — transposed LN1 output, c-tile pairs
        hT2 = {
            (cp, t2): big.tile(
                [P, 2, NOWN], F8, tag=f"hT{cp}_{t2}", name=f"hT{cp}_{t2}"
            )
            for cp in range(CP)
            for t2 in range(2)
        }
        # kT[(ft, t2)]: [P, 512] bf16 (QK stays bf16)
        kT = {
            (ft, t2): big.tile(
                [P, NOWN], BF16, tag=f"kT{ft}_{t2}", name=f"kT{ft}_{t2}"
            )
            for ft in range(CT)
            for t2 in range(2)
        }
        qT = [
            big.tile([P, NOWN], BF16, tag=f"qT{ft}", name=f"qT{ft}")
            for ft in range(CT)
        ]
        # vh[t2]: [P, 4, HEADS, VPAD] fp8 — V in natural token layout,
        # per-head padded; col DH holds 1.0 (softmax denominator trick)
        vh = [
            big.tile([P, QT, HEADS, VPAD], F8, tag=f"vh{t2}", name=f"vh{t2}")
            for t2 in range(2)
        ]
        # oT2[fp]: [P, 2, 512] fp8 — attention output, feature-tile pairs
        oT2 = [
            big.tile([P, 2, NOWN], F8, tag=f"oT{fp}", name=f"oT{fp}")
            for fp in range(CP)
        ]
        h2T2 = [
            big.tile([P, 2, NOWN], mf1, tag=f"h2T{cp}", name=f"h2T{cp}")
            for cp in range(CP)
        ]
        actT2 = [
            big.tile([P, 2, NOWN], mf2, tag=f"aT{hp}", name=f"aT{hp}")
            for hp in range(HP)
        ]

        for t2 in range(2):
            nc.vector.memset(vh[t2][:], 0.0)
            nc.vector.memset(vh[t2][:, :, :, DH : DH + 1], 1.0)

        # ================= Phase 1: LN1 -> hT2 =================
        with (
            tc.tile_pool(name="ln1", bufs=2) as ln1,
            tc.tile_pool(name="xtmp", bufs=2) as xtmp,
        ):
            for t in range(TT):
                if t < QT:
                    xt = x_own[t]
                else:
                    xt = xtmp.tile([P, C], F32, tag="xt", name="xt")
                    nc.sync.dma_start(xt, x_d[t * P : (t + 1) * P, :])
                h = layernorm_tile(ln1, xt)
                t2, tb = t // QT, t % QT
                ps = psum_wide.tile([P, C], F32R, tag="w", name=f"trp{t}")
                for ft in range(CT):
                    nc.tensor.transpose(
                        ps[:, ft * P : (ft + 1) * P],
                        h[:, ft * P : (ft + 1) * P],
                        ident,
                    )
                for cp in range(CP):
                    nc.any.tensor_copy(
                        out=hT2[(cp, t2)][:, :, tb * P : (tb + 1) * P],
                        in_=ps[:, cp * 2 * P : (cp + 1) * 2 * P].rearrange(
                            "p (two f) -> p two f", two=2
                        ),
                    )

        # ================= Phase 2: QKV =================
        HB = HEADS // 2  # heads per 512-wide V block
        with tc.tile_pool(name="qkv_psum", bufs=4, space="PSUM") as qkv_psum:

            def kq_sweep(t2, wslab, bT, sT, dst):
                """K or Q: transposed-output sweep; dst[ft] <- [P,512] bf16"""
                for ft in range(CT):
                    ps = qkv_psum.tile([P, NOWN], F32, tag="kvps", name="kvps")
                    for cp in range(CP):
                        nc.tensor.matmul(
                            ps,
                            lhsT=wslab[:, ft, 2 * cp : 2 * cp + 2, :],
                            rhs=hT2[(cp, t2)],
                            start=(cp == 0),
                            stop=(cp == CP - 1),
                            perf_mode=DROW,
                        )
                    nc.vector.tensor_scalar(
                        out=dst[ft],
                        in0=ps,
                        scalar1=sT[:, ft : ft + 1],
                        scalar2=bT[:, ft : ft + 1],
                        op0=OP.mult,
                        op1=OP.add,
                    )

            def v_sweep(t2):
                """V in natural layout: h stationary, vw moving."""
                for tb in range(QT):
                    ps = psum_wide.tile(
                        [P, HEADS, DH], F32, tag="w", name=f"vps{t2}_{tb}"
                    )
                    for cp in range(CP):
                        for blk in range(2):
                            nc.tensor.matmul(
                                ps[:, blk * HB : (blk + 1) * HB, :],
                                lhsT=hT2[(cp, t2)][
                                    :, :, tb * P : (tb + 1) * P
                                ],
                                rhs=vwslab[
                                    :,
                                    2 * cp : 2 * cp + 2,
                                    blk * 512 : (blk + 1) * 512,
                                ],
                                start=(cp == 0),
                                stop=(cp == CP - 1),
                                perf_mode=DROW,
                            )
                    nc.any.tensor_tensor(
                        vh[t2][:, tb, :, :DH],
                        ps,
                        vg_bc[:, :].rearrange("p (h d) -> p h d", h=HEADS),
                        OP.mult,
                    )

            kq_sweep(0, kwslab, kbT, ksT, [kT[(f, 0)] for f in range(CT)])
            v_sweep(0)
            kq_sweep(0, qwslab, qbT, qsT, qT)
            kq_sweep(1, kwslab, kbT, ksT, [kT[(f, 1)] for f in range(CT)])
            v_sweep(1)

        # ================= Phase 3: attention =================
        with (
            tc.tile_pool(name="attn", bufs=2) as attn_pool,
            tc.tile_pool(name="attn_ot", bufs=2, space="PSUM") as attn_ot,
        ):
            for h in range(HEADS):
                prow = (h % 2) * DH
                ftile = h // 2
                p_sb = attn_pool.tile([P, TT, NOWN], F8, tag="p_sb", name="p")
                for cp in range(CP):
                    st = psum_wide.tile(
                        [P, 2, NOWN], F32, tag="w", name=f"st{h}_{cp}"
                    )
                    for j in range(2):
                        c = cp * 2 + j
                        kv_slice = kT[(ftile, c // QT)][
                            prow : prow + DH, (c % QT) * P : (c % QT + 1) * P
                        ]
                        nc.tensor.matmul(
                            st[:, j, :],
                            lhsT=kv_slice,
                            rhs=qT[ftile][prow : prow + DH, :],
                            start=True,
                            stop=True,
                        )
                    # p = exp(SCALE * s)   (fp8 out)
                    nc.scalar.activation(
                        out=p_sb[:, 2 * cp : 2 * cp + 2, :],
                        in_=st,
                        func=AF.Exp,
                        scale=SCALE,
                    )
                ot = attn_ot.tile([P, NOWN], F32, tag="ot", name="ot")
                for cp in range(CP):
                    t2, c2 = cp // 2, cp % 2
                    nc.tensor.matmul(
                        ot[: DH + 1, :],
                        lhsT=vh[t2][:, 2 * c2 : 2 * c2 + 2, h, : DH + 1],
                        rhs=p_sb[:, 2 * cp : 2 * cp + 2, :],
                        start=(cp == 0),
                        stop=(cp == CP - 1),
                        perf_mode=DROW,
                    )
                # softmax denominators arrive in row DH (ones column of vh)
                rs = attn_pool.tile([1, NOWN], F32, tag="rs", name="rs")
                nc.vector.reciprocal(out=rs, in_=ot[DH : DH + 1, :])
                rsb = attn_pool.tile([DH, NOWN], F32, tag="rsb", name="rsb")
                nc.gpsimd.partition_broadcast(rsb, rs)
                nc.any.tensor_tensor(
                    oT2[ftile // 2][prow : prow + DH, ftile % 2, :],
                    ot[:DH, :],
                    rsb,
                    OP.mult,
                )

        # ================= Phase 4: proj + residual -> x2 (in place) ========
        # fold pb into x_own first (x_own already consumed by LN1; tile deps
        # order this correctly)
        for tq in range(QT):
            nc.any.tensor_tensor(x_own[tq], x_own[tq], pb_bc, OP.add)
        with tc.tile_pool(name="proj_ps", bufs=2, space="PSUM") as proj_ps:
            for ns in range(2):
                nsl = slice(ns * 512, (ns + 1) * 512)
                for tq in range(QT):
                    ps = proj_ps.tile([P, 512], F32, tag="pps", name="pps")
                    for fp in range(CP):
                        nc.tensor.matmul(
                            ps,
                            lhsT=oT2[fp][:, :, tq * P : (tq + 1) * P],
                            rhs=pslab[:, 2 * fp : 2 * fp + 2, nsl],
                            start=(fp == 0),
                            stop=(fp == CP - 1),
                            perf_mode=DROW,
                        )
                    nc.vector.tensor_tensor(ps, ps, psinv_bc[:, nsl], OP.mult)
                    nc.vector.tensor_tensor(
                        x2[tq][:, nsl], ps, x_own[tq][:, nsl], OP.add
                    )

        # ================= Phase 5: LN2 -> h2T2 =================
        with tc.tile_pool(name="ln2", bufs=2) as ln2:
            for t in range(QT):
                h = layernorm_tile(ln2, x2[t])
                ps = psum_wide.tile([P, C], F32R, tag="w", name=f"tr2{t}")
                for ft in range(CT):
                    nc.tensor.transpose(
                        ps[:, ft * P : (ft + 1) * P],
                        h[:, ft * P : (ft + 1) * P],
                        ident,
                    )
                for cp in range(CP):
                    nc.any.tensor_copy(
                        out=h2T2[cp][:, :, t * P : (t + 1) * P],
                        in_=ps[:, cp * 2 * P : (cp + 1) * 2 * P].rearrange(
                            "p (two f) -> p two f", two=2
                        ),
                    )

        # ================= Phase 6: FC1 + gelu -> actT2 =================
        with (
            tc.tile_pool(name="f1c", bufs=8) as f1c,
            tc.tile_pool(name="f1_ps", bufs=4, space="PSUM") as f1_ps,
        ):
            for hf in range(HT):
                ps = f1_ps.tile([P, NOWN], F32, tag="f1ps", name="f1ps")
                slab = f1c.tile([P, CT, P], mf1, tag="f1w", name="f1slab")
                nc.sync.dma_start(slab, io["f1w"][:, hf])
                if FC1_FP8:
                    for cp in range(CP):
                        nc.tensor.matmul(
                            ps,
                            lhsT=slab[:, 2 * cp : 2 * cp + 2, :],
                            rhs=h2T2[cp],
                            start=(cp == 0),
                            stop=(cp == CP - 1),
                            perf_mode=DROW,
                        )
                else:
                    for c in range(CT):
                        nc.tensor.matmul(
                            ps,
                            lhsT=slab[:, c, :],
                            rhs=h2T2[c // 2][:, c % 2, :],
                            start=(c == 0),
                            stop=(c == CT - 1),
                        )
                # gelu(ps * s + b), fused dequant+bias via activation
                nc.scalar.activation(
                    out=actT2[hf // 2][:, hf % 2, :],
                    in_=ps,
                    func=AF.Gelu,
                    bias=f1bT[:, hf : hf + 1],
                    scale=f1sT[:, hf : hf + 1],
                )

        # ================= Phase 7: FC2 + residual -> out =================
        with (
            tc.tile_pool(name="f2c", bufs=3) as f2c,
            tc.tile_pool(name="f2_ps", bufs=1, space="PSUM") as f2_ps,
            tc.tile_pool(name="out_sb", bufs=2) as out_pool,
        ):
            for ns in range(2):
                nsl = slice(ns * 512, (ns + 1) * 512)
                pss = [
                    f2_ps.tile([P, 512], F32, tag=f"f2ps{tq}", name=f"f2ps{tq}")
                    for tq in range(QT)
                ]
                NG = 4  # hidden-tile groups per DMA chunk
                for g in range(HT // NG):
                    gw = f2c.tile([P, NG, 512], mf2, tag="f2w", name=f"f2wg{g}")
                    nc.sync.dma_start(
                        gw, io["f2w"][:, ns, g * NG : (g + 1) * NG, :]
                    )
                    for tq in range(QT):
                        if FC2_FP8:
                            for i in range(NG // 2):
                                hp = (g * NG) // 2 + i
                                nc.tensor.matmul(
                                    pss[tq],
                                    lhsT=actT2[hp][
                                        :, :, tq * P : (tq + 1) * P
                                    ],
                                    rhs=gw[:, 2 * i : 2 * i + 2, :],
                                    start=(g == 0 and i == 0),
                                    stop=(
                                        g == HT // NG - 1 and i == NG // 2 - 1
                                    ),
                                    perf_mode=DROW,
                                )
                        else:
                            for i in range(NG):
                                hc = g * NG + i
                                nc.tensor.matmul(
                                    pss[tq],
                                    lhsT=actT2[hc // 2][
                                        :, hc % 2, tq * P : (tq + 1) * P
                                    ],
                                    rhs=gw[:, i, :],
                                    start=(hc == 0),
                                    stop=(hc == HT - 1),
                                )
                for tq in range(QT):
                    ot2 = out_pool.tile([P, 512], F32, tag="out_t", name="o")
                    nc.vector.tensor_tensor(
                        ot2, pss[tq], f2sinv_bc[:, nsl], OP.mult
                    )
                    nc.vector.tensor_tensor(ot2, ot2, f2b_bc[:, nsl], OP.add)
                    nc.vector.tensor_tensor(ot2, ot2, x2[tq][:, nsl], OP.add)
                    nc.sync.dma_start(out_d[tq * P : (tq + 1) * P, nsl], ot2)


_PROGRAM = None


def _get_program():
    global _PROGRAM
    if _PROGRAM is None:
        _PROGRAM = build_program()
    return _PROGRAM


def _quant_cols(w, dtype):
    """per-output-column absmax quantization; returns (w_q, dequant_scales)"""
    import ml_dtypes

    w = np.asarray(w, np.float64)
    if dtype == "fp8":
        amax = np.abs(w).max(axis=0)
        amax = np.where(amax == 0, 1.0, amax)
        s = E4M3_MAX / amax
        wq = np.clip(w * s, -E4M3_MAX, E4M3_MAX).astype(ml_dtypes.float8_e4m3)
        return wq, (1.0 / s).astype(np.float32)
    else:
        wq = w.astype(ml_dtypes.bfloat16)
        return wq, np.ones(w.shape[1], np.float32)


def build_in_maps(inputs):
    import ml_dtypes

    x = np.asarray(inputs["x"], np.float32)  # [4, 1024, 1024]
    ln1_g = np.asarray(inputs["ln1_g"], np.float64)
    ln1_b = np.asarray(inputs["ln1_b"], np.float64)
    ln2_g = np.asarray(inputs["ln2_g"], np.float64)
    ln2_b = np.asarray(inputs["ln2_b"], np.float64)
    qkv_w = np.asarray(inputs["qkv_w"], np.float64)
    qkv_b = np.asarray(inputs["qkv_b"], np.float64)
    proj_w = np.asarray(inputs["proj_w"], np.float64)
    proj_b = np.asarray(inputs["proj_b"], np.float64)
    fc1_w = np.asarray(inputs["fc1_w"], np.float64)
    fc1_b = np.asarray(inputs["fc1_b"], np.float64)
    fc2_w = np.asarray(inputs["fc2_w"], np.float64)
    fc2_b = np.asarray(inputs["fc2_b"], np.float64)

    # Fold LN affine into the following matmul:
    #   (xhat*g + b) @ W == xhat @ (diag(g) W) + b @ W
    qkv_w_f = ln1_g[:, None] * qkv_w
    qkv_b_f = qkv_b + ln1_b @ qkv_w
    f1w_f = ln2_g[:, None] * fc1_w
    f1b_f = fc1_b + ln2_b @ fc1_w

    qw = qkv_w_f[:, :C]
    kw = qkv_w_f[:, C : 2 * C]
    vw = qkv_w_f[:, 2 * C :]
    vb = qkv_b_f[2 * C :]

    # --- Q/K: per-column fp8 quant, dequant scale applied on chip ---
    qw8, qsinv = _quant_cols(qw, "fp8")
    kw8, ksinv = _quant_cols(kw, "fp8")

    # --- V: per-column fp8 quant; on-chip the psum is rescaled by vg so the
    # fp8 V tile holds v*t with t = 24/||vw_col||; t and the v bias both fold
    # into the proj weights/bias ---
    vw8, vsinv = _quant_cols(vw, "fp8")
    vnorm = np.linalg.norm(vw, axis=0)
    vnorm = np.where(vnorm == 0, 1.0, vnorm)
    t_v = 24.0 / vnorm
    vg = (vsinv * t_v).astype(np.float32)  # psum -> fp8 V scaling

    # --- proj: fold t_v and v bias; per-column fp8 quant ---
    pw_eff = proj_w / t_v[:, None]
    pb_eff = proj_b + vb @ proj_w
    pw8, psinv = _quant_cols(pw_eff, "fp8")

    # --- fc1 / fc2 ---
    f1w8, f1sinv = _quant_cols(f1w_f, "fp8" if FC1_FP8 else "bf16")
    f2w8, f2sinv = _quant_cols(fc2_w, "fp8" if FC2_FP8 else "bf16")

    # --- permute weights into SBUF slab layouts ---
    # q/k: [p, ft, c, f] from w[c*128+p, ft*128+f]
    def perm_kq(w8):
        return np.ascontiguousarray(
            w8.reshape(CT, P, CT, P).transpose(1, 2, 0, 3)
        )

    # v/proj: [p, c, n] from w[c*128+p, n]
    def perm_cn(w8):
        return np.ascontiguousarray(w8.reshape(CT, P, C).transpose(1, 0, 2))

    # fc1: [p, hf, c, f] from w[c*128+p, hf*128+f]
    f1wP = np.ascontiguousarray(
        f1w8.reshape(CT, P, HT, P).transpose(1, 2, 0, 3)
    )
    # fc2: [p, ns, hc, n] from w[hc*128+p, ns*512+n]
    f2wP = np.ascontiguousarray(
        f2w8.reshape(HT, P, 2, NOWN).transpose(1, 2, 0, 3)
    )

    def tbias(b):  # [n*128] -> [128, n] per-partition layout
        return np.ascontiguousarray(
            np.asarray(b, np.float32).reshape(-1, P).T
        )

    common = dict(
        qw=perm_kq(qw8),
        kw=perm_kq(kw8),
        vw=perm_cn(vw8),
        pw=perm_cn(pw8),
        f1w=f1wP,
        f2w=f2wP,
        qbt=tbias(qkv_b_f[:C]),
        kbt=tbias(qkv_b_f[C : 2 * C]),
        qst=tbias(qsinv),
        kst=tbias(ksinv),
        f1bt=tbias(f1b_f),
        f1st=tbias(f1sinv),
        vg=vg,
        psinv=psinv.astype(np.float32),
        pb=pb_eff.astype(np.float32),
        f2sinv=f2sinv.astype(np.float32),
        f2b=fc2_b.astype(np.float32),
    )
    in_maps = []
    for core in range(8):
        b, half = core // 2, core % 2
        own = x[b, half * NOWN : (half + 1) * NOWN, :]
        other = x[b, (1 - half) * NOWN : (2 - half) * NOWN, :]
        xp = np.ascontiguousarray(np.concatenate([own, other], axis=0))
        in_maps.append({**common, "x": xp})
    return in_maps


def kernel(**inputs):
    in_maps = build_in_maps(inputs)
    nc = _get_program()
    res = run_bass_kernel_spmd(nc, in_maps, core_ids=list(range(8)))
    outs = res.results

    y = np.empty((4, NTOK, C), np.float32)
    for core in range(8):
        b, half = core // 2, core % 2
        y[b, half * NOWN : (half + 1) * NOWN, :] = outs[core]["out"]
    return y


if __name__ == "__main__":
    prog = build_program()
    print("program built OK")
